# revision 4
# baseline (speedup 1.0000x reference)
"""Trainium2 Bass kernel for per-node temporal graph conv (LCN) — v2.

Math (matches the reference): for each node v with neighbor list idx[v]
(chain graph: v-1, v, v+1, masked at the ends),
    out[n,o,v,t] = b[v,o] + sum_{k,c,kt} x_pad[n,c,idx[v,k],t+kt] * Wm[v,o,c,k,kt]

Strategy: data-parallel over batch N across 8 cores (2 samples each);
weights/bias replicated. Host packs x into the SBUF node-pair layout in
bf16 so every device DMA is a large contiguous transfer.

Changes vs the original kernel (each validated against the NTFF
profile; note the PE clock plateaus at 2.0 or 2.4GHz per invocation —
device power state — so only like-for-like runs are comparable):
 - The two samples' matmuls are interleaved per weight slot (two PSUM
   banks per pair), so both samples stream against the just-landed
   weights/x blocks and the kernel's DMA order matches consumption.
 - 1-slot first weight chunk + staggered pair 0 (all of sample 0's
   mmi=0 taps first) so the PE starts as soon as 32KB of weights and
   one x block have landed, with no stall while sample 1's x arrives.
 - Single-block early x chunks: the DMA queues deliver ~50-100GB/s for
   the first ~3us after their first doorbell, and coarser chunks were
   measured to stall the PE >1us waiting for block 1.
 - Bias-adds alternate engines (sample 0 on Vector, sample 1 on the
   Scalar/Act engine) and y stores alternate HWDGE queues (scalar/sync)
   so the per-pair PSUM drain and the end-of-kernel chain run two-wide.
 - Per-pair y stores; the final pair is computed/stored in two column
   halves (first half mid-stream) and only 64 output partitions (single
   node), leaving a 32KB store as the end-of-kernel critical path.
 - Warm-up matmuls on sign-alternating (not zero) data sized to bridge
   from the earliest PE issue (~8us) to first-chunk arrival (~10us);
   the DVFS ramp to 2.4GHz keys off sustained PE activity and an idle
   gap re-arms it.

Per node pair (v=2m, 2m+1), outputs live on the PSUM partition dim
(128 = 2 nodes x 64 ch); two PSUM banks per pair (one per sample), 6
accumulating bf16 matmuls per bank. Bias fused into the PSUM->SBUF copy.
"""

import numpy as np
import ml_dtypes

import concourse.bacc as bacc
import concourse.mybir as mybir
from concourse.tile import TileContext, add_dep_helper
from concourse.bass_utils import run_bass_kernel_spmd

V, K, CIN, COUT, N, T, TK = 25, 3, 64, 64, 16, 512, 3
NCORES = 8
NPER = N // NCORES          # samples per core
TP = T + 2                  # block width incl. temporal zero pads
NB = (V + 1) // 2           # node-pair blocks
Y_BF16 = True               # store outputs as bf16 (host casts back)
WARM_MMS = 4                # PE warm-up matmuls (512 cols each)

_BF16 = mybir.dt.bfloat16
_F32 = mybir.dt.float32

_cache = {}


def _pair_taps(m):
    # mmi-major: the first three matmuls of pair m read only source block
    # m, so they can start before block m+1 has landed
    return [(kt, mmi) for mmi in range(2) for kt in range(TK) if 2 * m + mmi < V]


# slot index for each (m, kt, mmi), pair-major so weights stream in
# the same order the matmuls consume them
_SLOTS = {}
for _m in range(NB):
    for _t in _pair_taps(_m):
        _SLOTS[(_m,) + _t] = len(_SLOTS)
NSLOT = len(_SLOTS)  # 75


def _build_program():
    ydt = _BF16 if Y_BF16 else _F32
    nc = bacc.Bacc("TRN2", num_devices=NCORES)
    xp_in = nc.dram_tensor("xp", [NPER, 128, NB * TP], _BF16, kind="ExternalInput")
    wl_in = nc.dram_tensor("wl", [128, NSLOT * 128], _BF16, kind="ExternalInput")
    b_in = nc.dram_tensor("bias", [128, NB], _F32, kind="ExternalInput")
    y_out = nc.dram_tensor("y", [NPER, 128, NB * T], ydt, kind="ExternalOutput")
    warm_out = nc.dram_tensor("warm", [1, 4], _F32, kind="ExternalOutput")

    # weight chunks in slot units (pair m starts at slot 6m); 1-slot first
    # chunk: the staggered pair 0 needs only slot 0 to start
    WCHUNKS = [(0, 3), (3, 9)] + [
        (lo, min(lo + 8, NSLOT)) for lo in range(9, NSLOT, 8)
    ]
    # x chunks per sample, in block units; single-block chunks for blocks
    # 0-2 (consumed during the DMA-queue warm-up phase, where coarser
    # chunks were measured to stall the PE ~1.2us), then 2-block chunks
    XCHUNKS = [(0, 1), (1, 2), (2, 3)] + [
        (lo, min(lo + 2, NB)) for lo in range(3, NB, 2)
    ]
    # y store chunks per sample: one store per pair, issued as soon as its
    # bias lands — coarser chunks pile up 262KB+65KB of drain at the very
    # end.  Pair 12 is stored in two column halves (h0 mid-stream, h1 at
    # the end) so the end-of-kernel chain is bias+store of 65KB
    YCHUNKS = [(m, m + 1) for m in range(NB - 2)]

    prev_mm = [None]  # last-emitted matmul, to chain total PE order

    def chain(mi):
        # Chain every tensor-engine matmul to the previous one so the tile
        # scheduler keeps the PE stream in the emission order, which is
        # tuned to the DMA arrival schedule.
        if prev_mm[0] is not None:
            add_dep_helper(mi.ins, prev_mm[0].ins, reason="pin PE order")
        prev_mm[0] = mi
        return mi

    with TileContext(nc) as tc:
        with (
            tc.tile_pool(name="w", bufs=1) as wp,
            tc.tile_pool(name="x", bufs=1) as xp,
            tc.tile_pool(name="ps", bufs=8, space="PSUM") as pp,
            tc.tile_pool(name="o", bufs=1) as op,
        ):
            # PE HAM warm-up sized to end at/after first-chunk arrival
            # (~10.3us): an idle gap between dummies and real matmuls
            # re-arms the DVFS ramp window, so the dummies bridge from
            # ~8.3us (earliest PE issue after the Tile prologue) to the
            # first chunk's landing; real matmuls queue behind with no gap
            scratch = wp.tile([128, 640], _BF16, tag="scratch")
            warm_sb = wp.tile([1, 4], _F32, tag="warm_sb")
            wps = pp.tile([128, 512], _F32, tag="psA", bufs=4, name="warm_ps")
            # sign-alternating warm-up data: the DVFS activity monitor that
            # ramps the PE clock 1.2->2.4GHz responds to real switching
            # activity; all-zero operands draw almost no dynamic power
            nc.gpsimd.memset(scratch[:, 0:640:2], 1.0)
            nc.gpsimd.memset(scratch[:, 1:640:2], -1.0)
            for i in range(WARM_MMS):
                chain(
                    nc.tensor.matmul(
                        wps[:, :],
                        lhsT=scratch[:, 0:128],
                        rhs=scratch[:, 128:640],
                        start=(i == 0),
                        stop=(i == WARM_MMS - 1),
                    )
                )
            nc.vector.tensor_copy(out=warm_sb[:, :], in_=wps[0:1, 0:4])

            wl_sb = wp.tile([128, NSLOT * 128], _BF16, tag="wl")
            b_sb = wp.tile([128, NB], _F32, tag="bias")
            xs = [
                xp.tile([128, NB * TP], _BF16, tag=f"xs{n}", name=f"xs{n}")
                for n in range(NPER)
            ]
            ys = [
                op.tile([128, NB * T], ydt, tag=f"ys{n}", name=f"ys{n}")
                for n in range(NPER)
            ]

            # weights/bias on the scalar HWDGE queue, x loads on sync —
            # both in consumption order; the 8 DMA sem lanes self-throttle
            for ci, (lo, hi) in enumerate(WCHUNKS):
                nc.scalar.dma_start(
                    out=wl_sb[:, lo * 128 : hi * 128], in_=wl_in[:, lo * 128 : hi * 128]
                )
                if ci == 1:
                    # bias (6.6KB) right after the pair-0 weight chunks:
                    # any later and it queues behind ~1MB of weights,
                    # landing after pair 0's first PSUM copy needs it
                    nc.scalar.dma_start(out=b_sb[:, :], in_=b_in[:, :])
            # x chunks interleaved s0/s1 to match the interleaved matmuls
            for lo, hi in XCHUNKS:
                for n in range(NPER):
                    nc.sync.dma_start(
                        out=xs[n][:, lo * TP : hi * TP],
                        in_=xp_in[n, :, lo * TP : hi * TP],
                    )
            nc.sync.dma_start(out=warm_out[:, :], in_=warm_sb[:, :])

            def emit_group(m, c0, c1, uniq, stagger=False):
                # matmuls+bias+store for columns [c0, c1) of pair m;
                # sample 0 bias on Vector / store on scalar queue, sample 1
                # bias on Scalar(Act) / store on sync queue.  The last pair
                # is a single node: only output partitions 0-63 are live
                # (the host unpack ignores the rest), so its matmuls load
                # 64-wide stationary tiles and its bias/stores halve
                w = c1 - c0
                rows = 64 if m == NB - 1 else 128
                psA = pp.tile([128, w], _F32, tag="psA", bufs=4, name=f"psA{uniq}")
                psB = pp.tile([128, w], _F32, tag="psB", bufs=4, name=f"psB{uniq}")
                ps = [psA, psB]
                taps = _pair_taps(m)

                def mmit(n, kt, mmi, start, stop):
                    slot = _SLOTS[(m, kt, mmi)]
                    col = (m + mmi) * TP + kt + c0
                    return chain(
                        nc.tensor.matmul(
                            ps[n][0:rows, :],
                            lhsT=wl_sb[:, slot * 128 : slot * 128 + rows],
                            rhs=xs[n][:, col : col + w],
                            start=start,
                            stop=stop,
                        )
                    )

                if stagger:
                    # pair 0 only: sample 1's x block 0 lands ~1.8us after
                    # sample 0's, so run all of s0's mmi=0 taps first (own
                    # weight load each) instead of stalling the PE on the
                    # interleave; mmi=1 taps interleave/share as usual
                    for n in range(2):
                        for j in range(TK):
                            mmit(n, j, 0, start=(j == 0), stop=False)
                    for kt in range(TK):
                        mmit(0, kt, 1, False, kt == TK - 1)
                        mmit(1, kt, 1, False, kt == TK - 1)
                else:
                    for i, (kt, mmi) in enumerate(taps):
                        mmit(0, kt, mmi, i == 0, i == len(taps) - 1)
                        mmit(1, kt, mmi, i == 0, i == len(taps) - 1)
                # bias-adds split across engines so the tail (and the
                # per-pair PSUM drain) runs s0/s1 in parallel: s0 on
                # Vector, s1 on Scalar(Act).  One engine for both delays
                # every s1 store behind both biases (+2us tail, measured)
                nc.vector.tensor_scalar_add(
                    out=ys[0][0:rows, m * T + c0 : m * T + c1],
                    in0=psA[0:rows, :],
                    scalar1=b_sb[0:rows, m : m + 1],
                )
                nc.scalar.add(
                    ys[1][0:rows, m * T + c0 : m * T + c1],
                    psB[0:rows, :],
                    b_sb[0:rows, m : m + 1],
                )

            def store(lo_col, hi_col, rows=128):
                nc.scalar.dma_start(
                    out=y_out[0, 0:rows, lo_col:hi_col],
                    in_=ys[0][0:rows, lo_col:hi_col],
                )
                nc.sync.dma_start(
                    out=y_out[1, 0:rows, lo_col:hi_col],
                    in_=ys[1][0:rows, lo_col:hi_col],
                )

            LAST = NB - 1
            HSPLIT = 256
            ci = 0
            for m in range(LAST - 1):
                emit_group(m, 0, T, m, stagger=(m == 0))
                if ci < len(YCHUNKS) and m + 1 == YCHUNKS[ci][1]:
                    lo, hi = YCHUNKS[ci]
                    store(lo * T, hi * T)
                    ci += 1
                if m == 6:
                    # first column half of the final single-node pair,
                    # computed (and stored) mid-stream
                    emit_group(LAST, 0, HSPLIT, "l12h0")
                    store(LAST * T, LAST * T + HSPLIT, rows=64)
                if m == 7:
                    # first column half of the second-to-last pair: its
                    # 131KB store otherwise issues right at stream end and
                    # dominates the final drain
                    emit_group(LAST - 1, 0, HSPLIT, "l11h0")
                    store((LAST - 1) * T, (LAST - 1) * T + HSPLIT)
            emit_group(LAST - 1, HSPLIT, T, "l11h1")
            store((LAST - 1) * T + HSPLIT, LAST * T)
            emit_group(LAST, HSPLIT, T, "l12h1")
            store(LAST * T + HSPLIT, (LAST + 1) * T, rows=64)

    nc.compile()

    return nc


def _prep_weights(W, b, idx, mask):
    W = np.asarray(W, np.float32)
    b = np.asarray(b, np.float32)
    idx = np.asarray(idx)
    mask = np.asarray(mask)
    Wm = np.where(mask[:, None, None, :, None], W, 0.0)  # [V,O,C,K,TK]
    W4 = np.zeros((V, V, COUT, CIN, TK), np.float32)
    for v in range(V):
        for k in range(K):
            if mask[v, k]:
                W4[v, idx[v, k]] = Wm[v, :, :, k, :]
    wl = np.zeros((128, NSLOT * 128), np.float32)
    for (m, kt, mmi), slot in _SLOTS.items():
        blk = m + mmi
        for uh, u in ((0, 2 * blk - 1), (1, 2 * blk)):
            for vloc in range(2):
                v = 2 * m + vloc
                if 0 <= u < V and v < V:
                    # lhsT[64*uh + c, 64*vloc + o] = W4[v,u,o,c,kt]
                    wl[
                        64 * uh : 64 * uh + 64,
                        slot * 128 + 64 * vloc : slot * 128 + 64 * vloc + 64,
                    ] = W4[v, u, :, :, kt].T
    bias = np.zeros((128, NB), np.float32)
    for m in range(NB):
        for vloc in range(2):
            if 2 * m + vloc < V:
                bias[64 * vloc : 64 * vloc + 64, m] = b[2 * m + vloc]
    return wl.astype(ml_dtypes.bfloat16), bias


def _pack_x(x):
    # x: [N, CIN, V, T] f32 -> [N, 128, NB, TP] bf16 in node-pair layout
    xb = x.astype(ml_dtypes.bfloat16)
    xp = np.zeros((N, 128, NB, TP), ml_dtypes.bfloat16)
    # even nodes 2j -> partitions 64-127, block j
    xp[:, 64:128, :, 1 : T + 1] = xb[:, :, 0::2, :]
    # odd nodes 2j-1 -> partitions 0-63, blocks 1..12
    xp[:, 0:64, 1:NB, 1 : T + 1] = xb[:, :, 1::2, :]
    return np.ascontiguousarray(xp.reshape(N, 128, NB * TP))


def _unpack_y(yp):
    # yp: [N, 128, NB*T] -> [N, COUT, V, T] f32.
    # Partition p = vloc*64 + o, column = m*T + t, value = out[n,o,2m+vloc,t].
    y4 = np.asarray(yp, np.float32).reshape(N, 2, COUT, NB, T)
    out = np.empty((N, COUT, V, T), np.float32)
    out[:, :, 0::2, :] = y4[:, 0]
    out[:, :, 1::2, :] = y4[:, 1, :, : V // 2]
    return out


def _make_in_maps(inputs):
    x = np.ascontiguousarray(np.asarray(inputs["x"], np.float32))
    wl, bias = _prep_weights(inputs["W"], inputs["b"], inputs["idx"], inputs["mask"])
    xp = _pack_x(x)
    return [
        {
            "xp": np.ascontiguousarray(xp[c * NPER : (c + 1) * NPER]),
            "wl": wl,
            "bias": bias,
        }
        for c in range(NCORES)
    ]


def kernel(x, W, b, idx, mask):
    if "nc" not in _cache:
        _cache["nc"] = _build_program()
    nc = _cache["nc"]
    in_maps = _make_in_maps({"x": x, "W": W, "b": b, "idx": idx, "mask": mask})
    res = run_bass_kernel_spmd(nc, in_maps, list(range(NCORES)))
    yp = np.concatenate([res.results[c]["y"] for c in range(NCORES)], axis=0)
    return _unpack_y(yp)
